# revision 3
# baseline (speedup 1.0000x reference)
"""Trainium2 Bass kernel v3 for NeighbourAssignment GNN message passing.

v3 over v2: matmul diet (96 -> 34 matmuls/tile) to unblock the PE
sequencer:
- logits matmul merged into psw matmul: rhs = [wl_int | ws_aug] (260 cols),
  psum blocks of 2 chunks (512-fp32 bank-aligned slots, 260 used).
- k-major edge order (slot = k*128 + node) makes chunk == k and
  partition == node, so the per-edge t term is a free-dim broadcast of
  node-major exp(t): a ~ f * exp(t) with one matmul + one Act exp per tile.
- softmax exp runs per psum block (Act, strided read of the 4 logit cols).
"""

import sys
import functools

sys.path.insert(0, "/opt/trn_rl_repo")

import numpy as np
import ml_dtypes

N = 50000
K = 16
C = 64
S = 4
OUT = 64
N_CORES = 8
P = 128
NPC = N // N_CORES            # 6250 nodes per core
NT = (NPC + P - 1) // P       # 49 node tiles per core
NPAD = NT * P
EPT = P * K                   # 2048 edges per node tile
NCHUNK = EPT // P             # 16 edge chunks per tile
SPLIT = 32767
ROWS_A = SPLIT + 1            # row 0 = zeros
ROWS_B = N - SPLIT + 1        # row 0 = zeros
EW = 128                      # gather row width (bf16) = 256B
WCOL = S * OUT + S            # 260 merged rhs cols
SLOT = 512                    # psum cols per chunk (bank aligned)


def _build_program(nt: int):
    import concourse.bass as bass
    import concourse.bacc as bacc
    import concourse.mybir as mybir
    import concourse.tile as tile
    from contextlib import ExitStack

    fp32 = mybir.dt.float32
    bf16 = mybir.dt.bfloat16
    i16 = mybir.dt.int16

    nc = bacc.Bacc("TRN2", num_devices=N_CORES, debug=False,
                   dynamic_dma_scratch_size=98304)

    srcA_d = nc.dram_tensor("srcA", [ROWS_A, EW], bf16, kind="ExternalInput")
    srcB_d = nc.dram_tensor("srcB", [ROWS_B, EW], bf16, kind="ExternalInput")
    pk_d = nc.dram_tensor("pk", [nt * P, 3 * P], i16, kind="ExternalInput")
    wt_d = nc.dram_tensor("wt_aug", [C + 1, S], bf16, kind="ExternalInput")
    wl2_d = nc.dram_tensor("wl2", [C + 1, WCOL], bf16, kind="ExternalInput")
    out_d = nc.dram_tensor("out", [P, nt * K * OUT], bf16, kind="ExternalOutput")

    Copy = mybir.ActivationFunctionType.Copy
    Exp = mybir.ActivationFunctionType.Exp
    AX = mybir.AxisListType.X
    MUL = mybir.AluOpType.mult
    ADD = mybir.AluOpType.add

    def bcast(ap, count, at=None):
        new = ap.ap.copy()
        if at is None:
            new.append([0, count])
        else:
            new.insert(at, [0, count])
        return bass.AP(ap.tensor, ap.offset, new)

    with tile.TileContext(nc) as tc, ExitStack() as ctx:
        const = ctx.enter_context(tc.tile_pool(name="const", bufs=1))
        sbg = ctx.enter_context(tc.tile_pool(name="sbg", bufs=4))
        sbi = ctx.enter_context(tc.tile_pool(name="sbi", bufs=3))
        sbt = ctx.enter_context(tc.tile_pool(name="sbt", bufs=3))
        sbb = ctx.enter_context(tc.tile_pool(name="sbb", bufs=5))
        sbo = ctx.enter_context(tc.tile_pool(name="sbo", bufs=3))
        ps_w = ctx.enter_context(tc.tile_pool(name="ps_w", bufs=2, space="PSUM"))

        wt_sb = const.tile([C + 1, S], bf16)
        nc.sync.dma_start(wt_sb[:], wt_d.ap()[:, :])
        wl2_sb = const.tile([C + 1, WCOL], bf16)
        nc.sync.dma_start(wl2_sb[:], wl2_d.ap()[:, :])
        # Warmups: one PE consumer per const producer so steady matmuls
        # never need more than one fresh sync wait.
        warm = ps_w.tile([P, 4 * SLOT], fp32, tag="psw")
        for wi, g in enumerate([wt_sb, wl2_sb]):
            nc.tensor.matmul(warm[0:1, wi:wi + 1], lhsT=g[0:1, 0:1],
                             rhs=g[0:1, 0:1], start=True, stop=True,
                             skip_group_check=True)

        for it in range(nt):
            pk_t = sbi.tile([P, 3 * P], i16, tag="pk")
            nc.sync.dma_start(pk_t[:], pk_d.ap()[it * P:(it + 1) * P, :])
            idxA_t = pk_t[:, 0:P]
            idxB_t = pk_t[:, P:2 * P]
            xT_t = pk_t[0:C + 1, 2 * P:3 * P].bitcast(bf16)

            gA = sbg.tile([P, EPT], bf16, tag="gA")
            nc.gpsimd.dma_gather(
                out_ap=gA[:, :].rearrange("p (a b) -> p a b", a=1),
                in_ap=srcA_d.ap()[:, :], idxs_ap=idxA_t,
                num_idxs=EPT, num_idxs_reg=EPT,
                elem_size=EW, transpose=True, single_packet=False)
            gB = sbg.tile([P, EPT], bf16, tag="gB")
            nc.gpsimd.dma_gather(
                out_ap=gB[:, :].rearrange("p (a b) -> p a b", a=1),
                in_ap=srcB_d.ap()[:, :], idxs_ap=idxB_t,
                num_idxs=EPT, num_idxs_reg=EPT,
                elem_size=EW, transpose=True, single_packet=False)

            # ---- t chain: k-major edge order makes chunk == k and
            # partition == node, so exp(t) broadcasts along free dims.
            tps = ps_w.tile([P, 4 * SLOT], fp32, tag="psw")
            nc.tensor.matmul(tps[:, 0:S], lhsT=xT_t, rhs=wt_sb[:, :],
                             start=True, stop=True, skip_group_check=True)
            E_bf = sbt.tile([P, S], bf16, tag="E")
            nc.scalar.activation(E_bf[:], tps[:, 0:S], Exp)

            # ---- psw blocks (4 chunks each) + per-block exp ----------
            f = sbt.tile([P, S * NCHUNK], bf16, tag="f")
            pswcs = []
            for pb in range(NCHUNK // 4):
                psw = ps_w.tile([P, 4 * SLOT], fp32, tag="psw")
                for cb in range(4):
                    c = 4 * pb + cb
                    ws_ = slice(cb * SLOT, cb * SLOT + WCOL)
                    ec = slice(c * P, (c + 1) * P)
                    nc.tensor.matmul(psw[:, ws_], lhsT=gA[0:C + 1, ec],
                                     rhs=wl2_sb[:, :], start=True, stop=False,
                                     skip_group_check=True)
                    nc.tensor.matmul(psw[:, ws_], lhsT=gB[0:C + 1, ec],
                                     rhs=wl2_sb[:, :], start=False, stop=True,
                                     skip_group_check=True)
                # logits cols 256:260 of each slot -> f slice
                nc.scalar.activation(
                    f[:, 16 * pb:16 * pb + 16].rearrange("p (c s) -> p c s", s=S),
                    psw[:, :].rearrange("p (c j) -> p c j", c=4)[:, :, S * OUT:S * OUT + S],
                    Exp)
                # y cols -> bf16 SBUF
                pswc = sbb.tile([P, 4 * S * OUT], bf16, tag="pswc")
                pswcs.append(pswc)
                nc.scalar.activation(
                    pswc[:, :].rearrange("p (c j) -> p c j", c=4),
                    psw[:, :].rearrange("p (c j) -> p c j", c=4)[:, :, 0:S * OUT],
                    Copy)

            # ---- softmax weights: a = f*EE / sum_s(f*EE) -------------
            m1 = sbt.tile([P, S * NCHUNK], bf16, tag="m1")
            nc.vector.tensor_tensor(
                out=m1[:, :].rearrange("p (c s) -> p c s", s=S),
                in0=f[:, :].rearrange("p (c s) -> p c s", s=S),
                in1=bcast(E_bf[:, :], NCHUNK, at=1), op=MUL)
            d = sbt.tile([P, NCHUNK], fp32, tag="d")
            nc.vector.tensor_reduce(
                d[:], m1[:, :].rearrange("p (c s) -> p c s", s=S),
                axis=AX, op=ADD)
            r = sbt.tile([P, NCHUNK], fp32, tag="r")
            nc.vector.reciprocal(r[:], d[:])
            a = sbt.tile([P, S * NCHUNK], bf16, tag="a")
            nc.vector.tensor_tensor(
                out=a[:, :].rearrange("p (c s) -> p c s", s=S),
                in0=m1[:, :].rearrange("p (c s) -> p c s", s=S),
                in1=bcast(r[:, :], S), op=MUL)

            # ---- combine per 4 chunks --------------------------------
            out_t = sbo.tile([P, K * OUT], bf16, tag="out")
            for qb in range(NCHUNK // 4):
                pswc = pswcs[qb]
                tmp = sbb.tile([P, 4 * S * OUT], bf16, tag="tmp")
                nc.vector.tensor_tensor(
                    out=tmp[:, :].rearrange("p (cb o s) -> p cb o s", cb=4, s=S),
                    in0=pswc[:, :].rearrange("p (cb o s) -> p cb o s", cb=4, s=S),
                    in1=bcast(a[:, 4 * S * qb:4 * S * (qb + 1)]
                              .rearrange("p (cb s) -> p cb s", cb=4), OUT, at=2),
                    op=MUL)
                u = sbb.tile([P, 4 * OUT * 2], bf16, tag="u")
                tmp4 = tmp[:, :].rearrange("p (cb o s) -> p cb o s", cb=4, s=S)
                nc.vector.tensor_tensor(
                    out=u[:, :].rearrange("p (cb o h) -> p cb o h", cb=4, h=2),
                    in0=tmp4[:, :, :, 0:2], in1=tmp4[:, :, :, 2:4], op=ADD)
                u4 = u[:, :].rearrange("p (cb o h) -> p cb o h", cb=4, h=2)
                nc.vector.tensor_tensor(
                    out=out_t[:, qb * 4 * OUT:(qb + 1) * 4 * OUT]
                        .rearrange("p (cb o) -> p cb o", cb=4).unsqueeze(3),
                    in0=u4[:, :, :, 0:1], in1=u4[:, :, :, 1:2], op=ADD)

            nc.sync.dma_start(
                out_d.ap()[:, it * K * OUT:(it + 1) * K * OUT], out_t[:])

    nc.compile()
    return nc


@functools.lru_cache(maxsize=2)
def _get_program(nt: int):
    return _build_program(nt)


def _wrap_idx(idx_flat: np.ndarray) -> np.ndarray:
    nt = idx_flat.shape[0] // EPT
    w = idx_flat.reshape(nt, EPT // 16, 16).transpose(0, 2, 1)
    w = np.broadcast_to(w[:, None, :, :], (nt, 8, 16, EPT // 16))
    return np.ascontiguousarray(w).reshape(nt * P, EPT // 16).astype(np.int16)


def _prep_shared(x_full, src, Wt, bt, Ws, bs, W_lin, b_lin):
    bf = ml_dtypes.bfloat16
    srcA = np.zeros((ROWS_A, EW), np.float32)
    srcA[1:, :C] = src[:SPLIT]
    srcA[1:, C] = 1.0
    srcB = np.zeros((ROWS_B, EW), np.float32)
    srcB[1:, :C] = src[SPLIT:]
    srcB[1:, C] = 1.0

    # wl2: cols 0:256 = W_lin/S in (o, s) interleave (+ b_lin/S bias row);
    # cols 256:260 = Ws (+ (bs + bt) bias row).
    wl2 = np.zeros((C + 1, WCOL), np.float32)
    wl2[:C, :S * OUT] = (np.transpose(W_lin, (1, 2, 0)) / S).reshape(C, OUT * S)
    wl2[C, :S * OUT] = (b_lin.T / S).reshape(OUT * S)
    wl2[:C, S * OUT:] = Ws
    wl2[C, S * OUT:] = bs + bt
    wt_aug = np.zeros((C + 1, S), np.float32)
    wt_aug[:C] = Wt
    return (srcA.astype(bf), srcB.astype(bf), wt_aug.astype(bf),
            wl2.astype(bf))


def kernel(x, src, neighbor_idx, Wt, bt, Ws, bs, W_lin, b_lin, _trace=False):
    from concourse import bass_utils
    bf = ml_dtypes.bfloat16

    x = np.asarray(x, np.float32)
    src = np.asarray(src, np.float32)
    neighbor_idx = np.asarray(neighbor_idx, np.int64)
    srcA, srcB, wt_aug, wl2 = _prep_shared(
        x, src, np.asarray(Wt, np.float32), np.asarray(bt, np.float32),
        np.asarray(Ws, np.float32), np.asarray(bs, np.float32),
        np.asarray(W_lin, np.float32), np.asarray(b_lin, np.float32))

    nc = _get_program(NT)

    in_maps = []
    for core in range(N_CORES):
        lo = core * NPC
        idx = np.zeros((NPAD, K), np.int64)
        idx[:NPC] = neighbor_idx[lo:lo + NPC]
        valid = np.zeros((NPAD, K), bool)
        valid[:NPC] = True
        # k-major slot order within each tile: slot = k*128 + node
        flat = idx.reshape(NT, P, K).transpose(0, 2, 1).reshape(-1)
        vflat = valid.reshape(NT, P, K).transpose(0, 2, 1).reshape(-1)
        idxA = np.where(vflat & (flat < SPLIT), flat + 1, 0)
        idxB = np.where(vflat & (flat >= SPLIT), flat - (SPLIT - 1), 0)

        xT = np.zeros((C + 1, NPAD), np.float32)
        xT[:C, :NPC] = x[lo:lo + NPC].T
        xT[C, :NPC] = 1.0

        # packed per-tile load: [idxA | idxB | xT-bitcast] (768B/partition)
        pk = np.zeros((NT, P, 3 * P), np.int16)
        pk[:, :, 0:P] = _wrap_idx(idxA).reshape(NT, P, P)
        pk[:, :, P:2 * P] = _wrap_idx(idxB).reshape(NT, P, P)
        xTb = xT.astype(bf).view(np.int16)              # [C+1, NPAD]
        pk[:, 0:C + 1, 2 * P:3 * P] = (
            xTb.reshape(C + 1, NT, P).transpose(1, 0, 2))
        in_maps.append({
            "srcA": srcA, "srcB": srcB,
            "pk": pk.reshape(NT * P, 3 * P),
            "wt_aug": wt_aug, "wl2": wl2,
        })

    res = bass_utils.run_bass_kernel_spmd(
        nc, in_maps, core_ids=list(range(N_CORES)), trace=_trace)

    out = np.empty((N, K, OUT), np.float32)
    for core in range(N_CORES):
        o = np.asarray(res.results[core]["out"]).astype(np.float32)
        # out[p, (t, k, o)]: node = t*128 + p
        o = o.reshape(P, NT, K, OUT).transpose(1, 0, 2, 3).reshape(NPAD, K, OUT)
        out[core * NPC:(core + 1) * NPC] = o[:NPC]
    if _trace:
        kernel._last_results = res
    return out


# revision 4
# speedup vs baseline: 1.0005x; 1.0005x over previous
"""Trainium2 Bass kernel v3 for NeighbourAssignment GNN message passing.

v3 over v2: matmul diet (96 -> 34 matmuls/tile) to unblock the PE
sequencer:
- logits matmul merged into psw matmul: rhs = [wl_int | ws_aug] (260 cols),
  psum blocks of 2 chunks (512-fp32 bank-aligned slots, 260 used).
- k-major edge order (slot = k*128 + node) makes chunk == k and
  partition == node, so the per-edge t term is a free-dim broadcast of
  node-major exp(t): a ~ f * exp(t) with one matmul + one Act exp per tile.
- softmax exp runs per psum block (Act, strided read of the 4 logit cols).
"""

import sys
import functools

sys.path.insert(0, "/opt/trn_rl_repo")

import numpy as np
import ml_dtypes

N = 50000
K = 16
C = 64
S = 4
OUT = 64
N_CORES = 8
P = 128
NPC = N // N_CORES            # 6250 nodes per core
NT = (NPC + P - 1) // P       # 49 node tiles per core
NPAD = NT * P
EPT = P * K                   # 2048 edges per node tile
NCHUNK = EPT // P             # 16 edge chunks per tile
SPLIT = 32767
ROWS_A = SPLIT + 1            # row 0 = zeros
ROWS_B = N - SPLIT + 1        # row 0 = zeros
EW = 128                      # gather row width (bf16) = 256B
WCOL = S * OUT + S            # 260 merged rhs cols
SLOT = 512                    # psum cols per chunk (bank aligned)


def _build_program(nt: int):
    import concourse.bass as bass
    import concourse.bacc as bacc
    import concourse.mybir as mybir
    import concourse.tile as tile
    from contextlib import ExitStack

    fp32 = mybir.dt.float32
    bf16 = mybir.dt.bfloat16
    i16 = mybir.dt.int16

    nc = bacc.Bacc("TRN2", num_devices=N_CORES, debug=False,
                   dynamic_dma_scratch_size=131072)

    srcA_d = nc.dram_tensor("srcA", [ROWS_A, EW], bf16, kind="ExternalInput")
    srcB_d = nc.dram_tensor("srcB", [ROWS_B, EW], bf16, kind="ExternalInput")
    pk_d = nc.dram_tensor("pk", [nt * P, 3 * P], i16, kind="ExternalInput")
    wt_d = nc.dram_tensor("wt_aug", [C + 1, S], bf16, kind="ExternalInput")
    wl2_d = nc.dram_tensor("wl2", [C + 1, WCOL], bf16, kind="ExternalInput")
    out_d = nc.dram_tensor("out", [P, nt * K * OUT], bf16, kind="ExternalOutput")

    Copy = mybir.ActivationFunctionType.Copy
    Exp = mybir.ActivationFunctionType.Exp
    AX = mybir.AxisListType.X
    MUL = mybir.AluOpType.mult
    ADD = mybir.AluOpType.add

    def bcast(ap, count, at=None):
        new = ap.ap.copy()
        if at is None:
            new.append([0, count])
        else:
            new.insert(at, [0, count])
        return bass.AP(ap.tensor, ap.offset, new)

    with tile.TileContext(nc) as tc, ExitStack() as ctx:
        const = ctx.enter_context(tc.tile_pool(name="const", bufs=1))
        sbg = ctx.enter_context(tc.tile_pool(name="sbg", bufs=5))
        sbi = ctx.enter_context(tc.tile_pool(name="sbi", bufs=3))
        sbt = ctx.enter_context(tc.tile_pool(name="sbt", bufs=3))
        sbb = ctx.enter_context(tc.tile_pool(name="sbb", bufs=5))
        sbo = ctx.enter_context(tc.tile_pool(name="sbo", bufs=3))
        ps_w = ctx.enter_context(tc.tile_pool(name="ps_w", bufs=2, space="PSUM"))

        wt_sb = const.tile([C + 1, S], bf16)
        nc.sync.dma_start(wt_sb[:], wt_d.ap()[:, :])
        wl2_sb = const.tile([C + 1, WCOL], bf16)
        nc.sync.dma_start(wl2_sb[:], wl2_d.ap()[:, :])
        # Warmups: one PE consumer per const producer so steady matmuls
        # never need more than one fresh sync wait.
        warm = ps_w.tile([P, 4 * SLOT], fp32, tag="psw")
        for wi, g in enumerate([wt_sb, wl2_sb]):
            nc.tensor.matmul(warm[0:1, wi:wi + 1], lhsT=g[0:1, 0:1],
                             rhs=g[0:1, 0:1], start=True, stop=True,
                             skip_group_check=True)

        for it in range(nt):
            pk_t = sbi.tile([P, 3 * P], i16, tag="pk")
            nc.sync.dma_start(pk_t[:], pk_d.ap()[it * P:(it + 1) * P, :])
            idxA_t = pk_t[:, 0:P]
            idxB_t = pk_t[:, P:2 * P]
            xT_t = pk_t[0:C + 1, 2 * P:3 * P].bitcast(bf16)

            gA = sbg.tile([P, EPT], bf16, tag="gA")
            nc.gpsimd.dma_gather(
                out_ap=gA[:, :].rearrange("p (a b) -> p a b", a=1),
                in_ap=srcA_d.ap()[:, :], idxs_ap=idxA_t,
                num_idxs=EPT, num_idxs_reg=EPT,
                elem_size=EW, transpose=True, single_packet=False)
            gB = sbg.tile([P, EPT], bf16, tag="gB")
            nc.gpsimd.dma_gather(
                out_ap=gB[:, :].rearrange("p (a b) -> p a b", a=1),
                in_ap=srcB_d.ap()[:, :], idxs_ap=idxB_t,
                num_idxs=EPT, num_idxs_reg=EPT,
                elem_size=EW, transpose=True, single_packet=False)

            # ---- t chain: k-major edge order makes chunk == k and
            # partition == node, so exp(t) broadcasts along free dims.
            tps = ps_w.tile([P, 4 * SLOT], fp32, tag="psw")
            nc.tensor.matmul(tps[:, 0:S], lhsT=xT_t, rhs=wt_sb[:, :],
                             start=True, stop=True, skip_group_check=True)
            E_bf = sbt.tile([P, S], bf16, tag="E")
            nc.scalar.activation(E_bf[:], tps[:, 0:S], Exp)

            # ---- psw blocks (4 chunks each) + per-block exp ----------
            f = sbt.tile([P, S * NCHUNK], bf16, tag="f")
            pswcs = []
            for pb in range(NCHUNK // 4):
                psw = ps_w.tile([P, 4 * SLOT], fp32, tag="psw")
                for cb in range(4):
                    c = 4 * pb + cb
                    ws_ = slice(cb * SLOT, cb * SLOT + WCOL)
                    ec = slice(c * P, (c + 1) * P)
                    nc.tensor.matmul(psw[:, ws_], lhsT=gA[0:C + 1, ec],
                                     rhs=wl2_sb[:, :], start=True, stop=False,
                                     skip_group_check=True)
                    nc.tensor.matmul(psw[:, ws_], lhsT=gB[0:C + 1, ec],
                                     rhs=wl2_sb[:, :], start=False, stop=True,
                                     skip_group_check=True)
                # logits cols 256:260 of each slot -> f slice
                nc.scalar.activation(
                    f[:, 16 * pb:16 * pb + 16].rearrange("p (c s) -> p c s", s=S),
                    psw[:, :].rearrange("p (c j) -> p c j", c=4)[:, :, S * OUT:S * OUT + S],
                    Exp)
                # y cols -> bf16 SBUF
                pswc = sbb.tile([P, 4 * S * OUT], bf16, tag="pswc")
                pswcs.append(pswc)
                nc.scalar.activation(
                    pswc[:, :].rearrange("p (c j) -> p c j", c=4),
                    psw[:, :].rearrange("p (c j) -> p c j", c=4)[:, :, 0:S * OUT],
                    Copy)

            # ---- softmax weights: a = f*EE / sum_s(f*EE) -------------
            m1 = sbt.tile([P, S * NCHUNK], bf16, tag="m1")
            nc.vector.tensor_tensor(
                out=m1[:, :].rearrange("p (c s) -> p c s", s=S),
                in0=f[:, :].rearrange("p (c s) -> p c s", s=S),
                in1=bcast(E_bf[:, :], NCHUNK, at=1), op=MUL)
            d = sbt.tile([P, NCHUNK], fp32, tag="d")
            nc.vector.tensor_reduce(
                d[:], m1[:, :].rearrange("p (c s) -> p c s", s=S),
                axis=AX, op=ADD)
            r = sbt.tile([P, NCHUNK], fp32, tag="r")
            nc.vector.reciprocal(r[:], d[:])
            a = sbt.tile([P, S * NCHUNK], bf16, tag="a")
            nc.vector.tensor_tensor(
                out=a[:, :].rearrange("p (c s) -> p c s", s=S),
                in0=m1[:, :].rearrange("p (c s) -> p c s", s=S),
                in1=bcast(r[:, :], S), op=MUL)

            # ---- combine per 4 chunks --------------------------------
            out_t = sbo.tile([P, K * OUT], bf16, tag="out")
            for qb in range(NCHUNK // 4):
                pswc = pswcs[qb]
                tmp = sbb.tile([P, 4 * S * OUT], bf16, tag="tmp")
                nc.vector.tensor_tensor(
                    out=tmp[:, :].rearrange("p (cb o s) -> p cb o s", cb=4, s=S),
                    in0=pswc[:, :].rearrange("p (cb o s) -> p cb o s", cb=4, s=S),
                    in1=bcast(a[:, 4 * S * qb:4 * S * (qb + 1)]
                              .rearrange("p (cb s) -> p cb s", cb=4), OUT, at=2),
                    op=MUL)
                u = sbb.tile([P, 4 * OUT * 2], bf16, tag="u")
                tmp4 = tmp[:, :].rearrange("p (cb o s) -> p cb o s", cb=4, s=S)
                nc.vector.tensor_tensor(
                    out=u[:, :].rearrange("p (cb o h) -> p cb o h", cb=4, h=2),
                    in0=tmp4[:, :, :, 0:2], in1=tmp4[:, :, :, 2:4], op=ADD)
                u4 = u[:, :].rearrange("p (cb o h) -> p cb o h", cb=4, h=2)
                nc.vector.tensor_tensor(
                    out=out_t[:, qb * 4 * OUT:(qb + 1) * 4 * OUT]
                        .rearrange("p (cb o) -> p cb o", cb=4).unsqueeze(3),
                    in0=u4[:, :, :, 0:1], in1=u4[:, :, :, 1:2], op=ADD)

            nc.sync.dma_start(
                out_d.ap()[:, it * K * OUT:(it + 1) * K * OUT], out_t[:])

    nc.compile()
    return nc


@functools.lru_cache(maxsize=2)
def _get_program(nt: int):
    return _build_program(nt)


def _wrap_idx(idx_flat: np.ndarray) -> np.ndarray:
    nt = idx_flat.shape[0] // EPT
    w = idx_flat.reshape(nt, EPT // 16, 16).transpose(0, 2, 1)
    w = np.broadcast_to(w[:, None, :, :], (nt, 8, 16, EPT // 16))
    return np.ascontiguousarray(w).reshape(nt * P, EPT // 16).astype(np.int16)


def _prep_shared(x_full, src, Wt, bt, Ws, bs, W_lin, b_lin):
    bf = ml_dtypes.bfloat16
    srcA = np.zeros((ROWS_A, EW), np.float32)
    srcA[1:, :C] = src[:SPLIT]
    srcA[1:, C] = 1.0
    srcB = np.zeros((ROWS_B, EW), np.float32)
    srcB[1:, :C] = src[SPLIT:]
    srcB[1:, C] = 1.0

    # wl2: cols 0:256 = W_lin/S in (o, s) interleave (+ b_lin/S bias row);
    # cols 256:260 = Ws (+ (bs + bt) bias row).
    wl2 = np.zeros((C + 1, WCOL), np.float32)
    wl2[:C, :S * OUT] = (np.transpose(W_lin, (1, 2, 0)) / S).reshape(C, OUT * S)
    wl2[C, :S * OUT] = (b_lin.T / S).reshape(OUT * S)
    wl2[:C, S * OUT:] = Ws
    wl2[C, S * OUT:] = bs + bt
    wt_aug = np.zeros((C + 1, S), np.float32)
    wt_aug[:C] = Wt
    return (srcA.astype(bf), srcB.astype(bf), wt_aug.astype(bf),
            wl2.astype(bf))


def kernel(x, src, neighbor_idx, Wt, bt, Ws, bs, W_lin, b_lin, _trace=False):
    from concourse import bass_utils
    bf = ml_dtypes.bfloat16

    x = np.asarray(x, np.float32)
    src = np.asarray(src, np.float32)
    neighbor_idx = np.asarray(neighbor_idx, np.int64)
    srcA, srcB, wt_aug, wl2 = _prep_shared(
        x, src, np.asarray(Wt, np.float32), np.asarray(bt, np.float32),
        np.asarray(Ws, np.float32), np.asarray(bs, np.float32),
        np.asarray(W_lin, np.float32), np.asarray(b_lin, np.float32))

    nc = _get_program(NT)

    in_maps = []
    for core in range(N_CORES):
        lo = core * NPC
        idx = np.zeros((NPAD, K), np.int64)
        idx[:NPC] = neighbor_idx[lo:lo + NPC]
        valid = np.zeros((NPAD, K), bool)
        valid[:NPC] = True
        # k-major slot order within each tile: slot = k*128 + node
        flat = idx.reshape(NT, P, K).transpose(0, 2, 1).reshape(-1)
        vflat = valid.reshape(NT, P, K).transpose(0, 2, 1).reshape(-1)
        idxA = np.where(vflat & (flat < SPLIT), flat + 1, 0)
        idxB = np.where(vflat & (flat >= SPLIT), flat - (SPLIT - 1), 0)

        xT = np.zeros((C + 1, NPAD), np.float32)
        xT[:C, :NPC] = x[lo:lo + NPC].T
        xT[C, :NPC] = 1.0

        # packed per-tile load: [idxA | idxB | xT-bitcast] (768B/partition)
        pk = np.zeros((NT, P, 3 * P), np.int16)
        pk[:, :, 0:P] = _wrap_idx(idxA).reshape(NT, P, P)
        pk[:, :, P:2 * P] = _wrap_idx(idxB).reshape(NT, P, P)
        xTb = xT.astype(bf).view(np.int16)              # [C+1, NPAD]
        pk[:, 0:C + 1, 2 * P:3 * P] = (
            xTb.reshape(C + 1, NT, P).transpose(1, 0, 2))
        in_maps.append({
            "srcA": srcA, "srcB": srcB,
            "pk": pk.reshape(NT * P, 3 * P),
            "wt_aug": wt_aug, "wl2": wl2,
        })

    res = bass_utils.run_bass_kernel_spmd(
        nc, in_maps, core_ids=list(range(N_CORES)), trace=_trace)

    out = np.empty((N, K, OUT), np.float32)
    for core in range(N_CORES):
        o = np.asarray(res.results[core]["out"]).astype(np.float32)
        # out[p, (t, k, o)]: node = t*128 + p
        o = o.reshape(P, NT, K, OUT).transpose(1, 0, 2, 3).reshape(NPAD, K, OUT)
        out[core * NPC:(core + 1) * NPC] = o[:NPC]
    if _trace:
        kernel._last_results = res
    return out
